# revision 11
# baseline (speedup 1.0000x reference)
"""Expert-parallel MoE (top-k routing + SwiGLU experts) for 8 Trainium2 cores.

Strategy
--------
- Host computes the (tiny) gate: logits = x @ gate_w (+ noise * noise_weight),
  top-k selection, sparse softmax weights.  0.03% of total FLOPs.
- Expert-parallel: core e owns expert e's weights.  Host gathers the tokens
  routed to expert e (padded to a common capacity C), core e runs a dense
  fused SwiGLU MLP over them:  out = (x@w1+b1) * silu(x@w2+b2) @ wp + bp,
  scaled by the per-token gate weight (folded into the final evacuation).
- Host scatter-adds the 8 partial outputs back to token positions
  (the "all-reduce of partial combined output" done at gather time).

Device kernel layout (all "transposed": tokens live on the free axis)
---------------------------------------------------------------------
- prologue: xe [C,D] --strided DMA--> SBUF --> xeT [D,C] (contiguous scratch)
- loop over 8 h-groups of 512 rows of H:
    stream w1/w2/wp slices for the group (large contiguous DMAs),
    loop over token blocks of 512:
      hT[128h, tok] = (w1g.T @ xT + b1) * silu(w2g.T @ xT + b2)   (PSUM acc over D)
      out_acc[128d, tok] += wpg.T @ hT                            (PSUM acc over 512h)
- epilogue: out = (out_acc + bp) * gate_weight, strided DMA to out [C,D]
"""

import sys
import numpy as np

sys.path.insert(0, "/opt/trn_rl_repo")

D = 1024
H = 4096
E = 8
KD = D // 128          # 8 k-tiles over D
G = 8                  # h-groups
HJ = 4                 # 128-row h-tiles per group (G*HJ*128 == H)
TB = 512               # token block

_NC_CACHE = {}


def _build(C, silu_native=True):
    import concourse.mybir as mybir
    import concourse.tile as tile
    from concourse import bacc

    f32 = mybir.dt.float32
    ACT = mybir.ActivationFunctionType
    ALU = mybir.AluOpType

    nc = bacc.Bacc()
    xeT = nc.dram_tensor("xeT", [D, C], f32, kind="ExternalInput")
    w1 = nc.dram_tensor("w1", [D, H], f32, kind="ExternalInput")
    w2 = nc.dram_tensor("w2", [D, H], f32, kind="ExternalInput")
    wp = nc.dram_tensor("wp", [H, D], f32, kind="ExternalInput")
    b1 = nc.dram_tensor("b1", [H], f32, kind="ExternalInput")
    b2 = nc.dram_tensor("b2", [H], f32, kind="ExternalInput")
    bp = nc.dram_tensor("bp", [D], f32, kind="ExternalInput")
    gwb = nc.dram_tensor("gwb", [128, C], f32, kind="ExternalInput")
    out = nc.dram_tensor("out", [C, D], f32, kind="ExternalOutput")

    blocks = []
    o = 0
    while o < C:
        blocks.append((o, min(TB, C - o)))
        o += TB

    # strided views
    xTr = xeT.rearrange("(kt p) c -> p kt c", p=128)              # [128,8,C]
    # w1/w2 halves: [gg, hh, p, k, c] with h-col = gg*512 + hh*256 + c
    w1r = w1.rearrange("(k p) (gg hh c) -> gg hh p k c", p=128, hh=2, c=256)
    w2r = w2.rearrange("(k p) (gg hh c) -> gg hh p k c", p=128, hh=2, c=256)
    # wp halves: rows h = gg*512 + hh*256 + hk*128 + p
    wpr = wp.rearrange("(gg hh hk p) c -> gg hh p hk c", p=128, hk=2, hh=2)
    outr = out.rearrange("c (dm p) -> dm p c", p=128)             # [8,128,C]
    b1r = b1.rearrange("(m p) -> p m", p=128)                     # [128,32]
    b2r = b2.rearrange("(m p) -> p m", p=128)
    bpr = bp.rearrange("(m p) -> p m", p=128)                     # [128,8]

    with tile.TileContext(nc) as tc:
        with (
            tc.tile_pool(name="pw13", bufs=3) as pw13,
            tc.tile_pool(name="pwp", bufs=3) as pwp,
            tc.tile_pool(name="px", bufs=2) as px,
            tc.tile_pool(name="pht", bufs=1) as pht,
            tc.tile_pool(name="ps2", bufs=1) as ps2,
            tc.tile_pool(name="pacc", bufs=1) as pacc,
            tc.tile_pool(name="pst", bufs=2) as pst,
            tc.tile_pool(name="pgw", bufs=1) as pgw,
            tc.tile_pool(name="pb", bufs=1) as pb,
            tc.tile_pool(name="pp", bufs=6, space="PSUM") as pp,
        ):
            b1s = pb.tile([128, G * HJ], f32, tag="b1s")
            nc.sync.dma_start(b1s[:], b1r)
            b2s = pb.tile([128, G * HJ], f32, tag="b2s")
            nc.sync.dma_start(b2s[:], b2r)
            bps = pb.tile([128, KD], f32, tag="bps")
            nc.sync.dma_start(bps[:], bpr)

            oacc = [pacc.tile([128, C], f32, tag=f"o{dm}", name=f"oacc{dm}")
                    for dm in range(KD)]

            # ---- main: h-groups of 512 ----
            for g in range(G):
                w1h, w2h, wph = [], [], []
                for hh in range(2):
                    t = pw13.tile([128, 8 * 256], f32, tag="w1s")
                    nc.sync.dma_start(
                        t[:].rearrange("p (k c) -> p k c", c=256), w1r[g, hh])
                    w1h.append(t)
                    t = pw13.tile([128, 8 * 256], f32, tag="w2s")
                    nc.sync.dma_start(
                        t[:].rearrange("p (k c) -> p k c", c=256), w2r[g, hh])
                    w2h.append(t)
                    t = pwp.tile([128, 2 * 1024], f32, tag="wps")
                    nc.sync.dma_start(
                        t[:].rearrange("p (hk c) -> p hk c", c=1024), wpr[g, hh])
                    wph.append(t)

                for (bo, bs) in blocks:
                    xall = px.tile([128, KD * bs], f32, tag="xall")
                    xv = xall[:].rearrange("p (kt c) -> p kt c", c=bs)
                    nc.sync.dma_start(xv, xTr[:, :, bo:bo + bs])

                    hts = []
                    for hj in range(HJ):
                        hm = g * HJ + hj
                        hh, co = hj // 2, (hj % 2) * 128
                        ps1 = pp.tile([128, bs], f32, tag="ps")
                        for k in range(KD):
                            nc.tensor.matmul(
                                ps1[:], w1h[hh][:, k * 256 + co: k * 256 + co + 128],
                                xv[:, k, :], start=(k == 0), stop=(k == KD - 1))
                        ps2t = pp.tile([128, bs], f32, tag="ps")
                        for k in range(KD):
                            nc.tensor.matmul(
                                ps2t[:], w2h[hh][:, k * 256 + co: k * 256 + co + 128],
                                xv[:, k, :], start=(k == 0), stop=(k == KD - 1))
                        s2 = ps2.tile([128, bs], f32, tag="s2")
                        if silu_native:
                            nc.scalar.activation(s2[:], ps2t[:], ACT.Silu,
                                                 bias=b2s[:, hm:hm + 1])
                        else:
                            sg = ps2.tile([128, bs], f32, tag="sg")
                            nc.scalar.activation(sg[:], ps2t[:], ACT.Sigmoid,
                                                 bias=b2s[:, hm:hm + 1])
                            nc.vector.scalar_tensor_tensor(
                                s2[:], ps2t[:], b2s[:, hm:hm + 1], sg[:],
                                op0=ALU.add, op1=ALU.mult)
                        ht = pht.tile([128, bs], f32, tag=f"h{hj}")
                        nc.vector.scalar_tensor_tensor(
                            ht[:], ps1[:], b1s[:, hm:hm + 1], s2[:],
                            op0=ALU.add, op1=ALU.mult)
                        hts.append(ht)

                    for dm in range(KD):
                        psB = pp.tile([128, bs], f32, tag="ps")
                        for hk in range(HJ):
                            hh, co = hk // 2, (hk % 2) * 1024
                            nc.tensor.matmul(
                                psB[:], wph[hh][:, co + dm * 128: co + dm * 128 + 128],
                                hts[hk][:], start=(hk == 0), stop=(hk == HJ - 1))
                        if g == 0:
                            nc.scalar.activation(oacc[dm][:, bo:bo + bs], psB[:],
                                                 ACT.Copy)
                        else:
                            nc.vector.tensor_add(oacc[dm][:, bo:bo + bs],
                                                 oacc[dm][:, bo:bo + bs], psB[:])

            # ---- epilogue: out = (out_acc + bp) * gate_weight ----
            for (bo, bs) in blocks:
                gwt = pgw.tile([128, bs], f32, tag="gw")
                nc.sync.dma_start(gwt[:], gwb[:, bo:bo + bs])
                for dm in range(KD):
                    ot = pst.tile([128, bs], f32, tag="ostage")
                    nc.vector.scalar_tensor_tensor(
                        ot[:], oacc[dm][:, bo:bo + bs], bps[:, dm:dm + 1], gwt[:],
                        op0=ALU.add, op1=ALU.mult)
                    nc.sync.dma_start(outr[dm][:, bo:bo + bs], ot[:])

    nc.finalize()
    return nc


def _route(x2d, noise2d, gate_w, noise_weight, kk):
    T = x2d.shape[0]
    logits = x2d @ gate_w
    logits = logits + noise2d * noise_weight[None, :]
    kk = int(kk)
    Ee = logits.shape[1]
    if kk >= Ee:
        sel = np.ones((T, Ee), dtype=bool)
    else:
        part = np.argpartition(-logits, kk - 1, axis=1)[:, :kk]
        sel = np.zeros((T, Ee), dtype=bool)
        sel[np.arange(T)[:, None], part] = True
    mx = logits.max(axis=1, keepdims=True)
    ex = np.exp(logits - mx, dtype=np.float32) * sel
    gw = ex / ex.sum(axis=1, keepdims=True)
    return sel, gw.astype(np.float32)


def kernel(**inputs):
    from concourse.bass_utils import run_bass_kernel_spmd

    x = np.asarray(inputs["x"], dtype=np.float32)
    noise = np.asarray(inputs["noise"], dtype=np.float32)
    gate_w = np.asarray(inputs["gate_w"], dtype=np.float32)
    noise_weight = np.asarray(inputs["noise_weight"], dtype=np.float32)
    w1 = np.asarray(inputs["w1"], dtype=np.float32)
    b1 = np.asarray(inputs["b1"], dtype=np.float32)
    w2 = np.asarray(inputs["w2"], dtype=np.float32)
    b2 = np.asarray(inputs["b2"], dtype=np.float32)
    wp = np.asarray(inputs["wp"], dtype=np.float32)
    bp = np.asarray(inputs["bp"], dtype=np.float32)
    kk = int(np.asarray(inputs["k"]))

    B, S, _ = x.shape
    T = B * S
    x2d = np.ascontiguousarray(x.reshape(T, D))
    noise2d = noise.reshape(T, E)

    sel, gw = _route(x2d, noise2d, gate_w, noise_weight, kk)
    idxs = [np.nonzero(sel[:, e])[0] for e in range(E)]
    maxn = max(len(i) for i in idxs)
    C = max(512, ((maxn + 127) // 128) * 128)

    if C not in _NC_CACHE:
        _NC_CACHE[C] = _build(C)
    nc = _NC_CACHE[C]

    in_maps = []
    for e in range(E):
        idx = idxs[e]
        n = len(idx)
        xeT = np.zeros((D, C), dtype=np.float32)
        xeT[:, :n] = x2d[idx].T
        gwb = np.zeros((128, C), dtype=np.float32)
        gwb[:, :n] = gw[idx, e][None, :]
        in_maps.append({
            "xeT": xeT,
            "w1": np.ascontiguousarray(w1[e]),
            "w2": np.ascontiguousarray(w2[e]),
            "wp": np.ascontiguousarray(wp[e]),
            "b1": np.ascontiguousarray(b1[e]),
            "b2": np.ascontiguousarray(b2[e]),
            "bp": np.ascontiguousarray(bp[e]),
            "gwb": gwb,
        })

    res = run_bass_kernel_spmd(nc, in_maps, core_ids=list(range(E))).results

    y2d = np.zeros((T, D), dtype=np.float32)
    for e in range(E):
        n = len(idxs[e])
        if n:
            y2d[idxs[e]] += res[e]["out"][:n]
    return y2d.reshape(B, S, D)
